# revision 6
# baseline (speedup 1.0000x reference)
"""2-layer GCN on 8 trn2 NeuronCores.

Full inputs in, full outputs out. Edges are sorted by dst on the host and
packed into groups of <=128 dst-nodes / <=2048 edges (16 tiles of 128).
Each core processes a contiguous run of groups. The per-tile segment-sum is
done as a TensorE matmul with an on-device-built one-hot*(norm) selection
matrix, accumulating 16 tiles per group in PSUM.

Three SPMD launches (host does only indexing/concat between them):
  A: S0 = X @ W0.T          (node-sharded, 1/8 per core)
  B: H  = relu(seg_sum(S0[src]*norm, dst))   (edge/group-sharded)
  C: Z  = seg_sum(H[src]*norm, dst) @ W1.T   (edge/group-sharded)
"""

import time

import numpy as np

import concourse.bacc as bacc
import concourse.bass as bass
import concourse.tile as tile
from concourse import mybir
from concourse.bass_utils import run_bass_kernel_spmd
from concourse.masks import make_identity

P = 128
TPG = 16                 # tiles (of 128 edges) per group
EPG = P * TPG            # 2048 edge slots per group
NCORES = 8
N = 50000
D = 128
F32 = mybir.dt.float32
I32 = mybir.dt.int32

LAST_TIMES = {}


def _pack_groups(dst_sorted):
    """Greedy pack sorted dst nodes into groups (<=P nodes, <=EPG edges).
    Returns list of (edge_start, edge_cnt, node_ids ndarray)."""
    nodes, counts = np.unique(dst_sorted, return_counts=True)
    groups = []
    i, e = 0, 0
    nn = len(nodes)
    while i < nn:
        es = e
        ns = i
        cnt_e = 0
        while i < nn and (i - ns) < P and cnt_e + counts[i] <= EPG:
            cnt_e += int(counts[i])
            i += 1
        assert i > ns, "single node exceeds group capacity"
        e += cnt_e
        groups.append((es, cnt_e, nodes[ns:i]))
    return groups


def _build_program_a(CH):
    nc = bacc.Bacc(None, target_bir_lowering=False)
    xin = nc.declare_dram_parameter("xin", [CH * P, D], F32, isOutput=False)
    w0t = nc.declare_dram_parameter("w0t", [D, D], F32, isOutput=False)
    s0out = nc.declare_dram_parameter("s0out", [CH * P, D], F32, isOutput=True)
    with tile.TileContext(nc) as tc:
        with (
            tc.tile_pool(name="const", bufs=1) as cpool,
            tc.tile_pool(name="sbuf", bufs=4) as pool,
            tc.tile_pool(name="psum", bufs=4, space="PSUM") as psum,
        ):
            ident = cpool.tile([P, P], dtype=F32)
            make_identity(nc, ident[:])
            w0t_sb = cpool.tile([D, D], dtype=F32)
            nc.sync.dma_start(out=w0t_sb[:], in_=w0t[:])
            for c in range(CH):
                x_sb = pool.tile([P, D], dtype=F32, tag="x")
                nc.sync.dma_start(out=x_sb[:], in_=xin[c * P:(c + 1) * P, :])
                xt_ps = psum.tile([P, P], dtype=F32, tag="xt")
                nc.tensor.transpose(out=xt_ps[:], in_=x_sb[:], identity=ident[:])
                xt_sb = pool.tile([P, P], dtype=F32, tag="xts")
                nc.vector.tensor_copy(xt_sb[:], xt_ps[:])
                s_ps = psum.tile([P, D], dtype=F32, tag="s")
                nc.tensor.matmul(out=s_ps[:], lhsT=xt_sb[:], rhs=w0t_sb[:],
                                 start=True, stop=True)
                s_sb = pool.tile([P, D], dtype=F32, tag="ss")
                nc.vector.tensor_copy(s_sb[:], s_ps[:])
                nc.sync.dma_start(out=s0out[c * P:(c + 1) * P, :], in_=s_sb[:])
    nc.compile()
    return nc


def _spmm_body(nc, tc, G, src_dram, idx, sn, iota, out_dram, relu, w1t):
    """Shared SpMM loop. If w1t is not None, apply (@ W1.T) per group."""
    with (
        tc.tile_pool(name="const", bufs=1) as cpool,
        tc.tile_pool(name="sbuf", bufs=4) as pool,
        tc.tile_pool(name="psum", bufs=2, space="PSUM") as psum,
        tc.tile_pool(name="psum2", bufs=2, space="PSUM") as psum2,
    ):
        iota_sb = cpool.tile([P, P], dtype=F32)
        nc.sync.dma_start(out=iota_sb[:], in_=iota[:])
        if w1t is not None:
            ident = cpool.tile([P, P], dtype=F32)
            make_identity(nc, ident[:])
            w1t_sb = cpool.tile([D, D], dtype=F32)
            nc.sync.dma_start(out=w1t_sb[:], in_=w1t[:])
        for g in range(G):
            idx_sb = pool.tile([P, TPG], dtype=I32, tag="idx")
            nc.sync.dma_start(out=idx_sb[:], in_=idx[g])
            sn_sb = pool.tile([P, 2 * TPG], dtype=F32, tag="sn")
            nc.sync.dma_start(out=sn_sb[:], in_=sn[g])
            acc_ps = psum.tile([P, D], dtype=F32, tag="acc")
            for t in range(TPG):
                g_sb = pool.tile([P, D], dtype=F32, tag="gat")
                nc.gpsimd.indirect_dma_start(
                    out=g_sb[:], out_offset=None, in_=src_dram[:],
                    in_offset=bass.IndirectOffsetOnAxis(ap=idx_sb[:, t:t + 1], axis=0),
                )
                sel = pool.tile([P, P], dtype=F32, tag="sel")
                nc.vector.tensor_tensor(
                    out=sel[:], in0=sn_sb[:, t:t + 1].to_broadcast([P, P])[:],
                    in1=iota_sb[:], op=mybir.AluOpType.is_equal,
                )
                pm = pool.tile([P, P], dtype=F32, tag="pm")
                nc.vector.tensor_scalar_mul(pm[:], sel[:], sn_sb[:, TPG + t:TPG + t + 1])
                nc.tensor.matmul(out=acc_ps[:], lhsT=pm[:], rhs=g_sb[:],
                                 start=(t == 0), stop=(t == TPG - 1))
            if w1t is None:
                h_sb = pool.tile([P, D], dtype=F32, tag="h")
                if relu:
                    nc.scalar.activation(h_sb[:], acc_ps[:],
                                         mybir.ActivationFunctionType.Relu)
                else:
                    nc.vector.tensor_copy(h_sb[:], acc_ps[:])
                nc.sync.dma_start(out=out_dram[g * P:(g + 1) * P, :], in_=h_sb[:])
            else:
                a_sb = pool.tile([P, D], dtype=F32, tag="a")
                nc.vector.tensor_copy(a_sb[:], acc_ps[:])
                at_ps = psum2.tile([P, P], dtype=F32, tag="at")
                nc.tensor.transpose(out=at_ps[:], in_=a_sb[:], identity=ident[:])
                at_sb = pool.tile([P, P], dtype=F32, tag="ats")
                nc.vector.tensor_copy(at_sb[:], at_ps[:])
                z_ps = psum2.tile([P, D], dtype=F32, tag="z")
                nc.tensor.matmul(out=z_ps[:], lhsT=at_sb[:], rhs=w1t_sb[:],
                                 start=True, stop=True)
                z_sb = pool.tile([P, D], dtype=F32, tag="zs")
                nc.vector.tensor_copy(z_sb[:], z_ps[:])
                nc.sync.dma_start(out=out_dram[g * P:(g + 1) * P, :], in_=z_sb[:])


def _build_program_bc(G, relu, with_w1):
    nc = bacc.Bacc(None, target_bir_lowering=False)
    src_t = nc.declare_dram_parameter("srct", [N, D], F32, isOutput=False)
    idx = nc.declare_dram_parameter("idx", [G, P, TPG], I32, isOutput=False)
    sn = nc.declare_dram_parameter("sn", [G, P, 2 * TPG], F32, isOutput=False)
    iota = nc.declare_dram_parameter("iota", [P, P], F32, isOutput=False)
    w1t = None
    if with_w1:
        w1t = nc.declare_dram_parameter("w1t", [D, D], F32, isOutput=False)
    out = nc.declare_dram_parameter("out", [G * P, D], F32, isOutput=True)
    with tile.TileContext(nc) as tc:
        _spmm_body(nc, tc, G, src_t, idx, sn, iota, out, relu, w1t)
    nc.compile()
    return nc


def kernel(X, W0, W1, norm, src, dst):
    t0 = time.perf_counter()
    X = np.asarray(X, dtype=np.float32)
    W0 = np.asarray(W0, dtype=np.float32)
    W1 = np.asarray(W1, dtype=np.float32)
    norm = np.asarray(norm, dtype=np.float32)
    src = np.asarray(src).astype(np.int64)
    dst = np.asarray(dst).astype(np.int64)
    E = src.shape[0]

    # ---- host preprocessing: sort by dst, pack groups, shard to cores ----
    order = np.argsort(dst, kind="stable")
    src_s = src[order].astype(np.int32)
    dst_s = dst[order]
    norm_s = norm[order]
    groups = _pack_groups(dst_s)
    Gtot = len(groups)
    # contiguous assignment balanced by edges
    cum = np.cumsum([g[1] for g in groups])
    core_of = np.minimum((8 * (cum - 1) // E).astype(np.int64), NCORES - 1)
    per_core = [[] for _ in range(NCORES)]
    for gi, g in enumerate(groups):
        per_core[int(core_of[gi])].append(g)
    G = max(len(lst) for lst in per_core)

    idx_arr = np.zeros((NCORES, G, P, TPG), dtype=np.int32)
    sn_arr = np.zeros((NCORES, G, P, 2 * TPG), dtype=np.float32)
    sn_arr[:, :, :, :TPG] = -1.0  # slot=-1 never matches iota -> zero row
    # assembly indexing: out_rows[core] -> global node ids
    asm_rows, asm_ids = [], []
    for c in range(NCORES):
        rows_l, ids_l = [], []
        for g_i, (es, ce, node_ids) in enumerate(per_core[c]):
            d_loc = np.searchsorted(node_ids, dst_s[es:es + ce]).astype(np.float32)
            j = np.arange(ce)
            t_i, p_i = j // P, j % P
            idx_arr[c, g_i, p_i, t_i] = src_s[es:es + ce]
            sn_arr[c, g_i, p_i, t_i] = d_loc
            sn_arr[c, g_i, p_i, TPG + t_i] = norm_s[es:es + ce]
            rows_l.append(g_i * P + np.arange(len(node_ids)))
            ids_l.append(node_ids)
        asm_rows.append(np.concatenate(rows_l) if rows_l else np.zeros(0, np.int64))
        asm_ids.append(np.concatenate(ids_l) if ids_l else np.zeros(0, np.int64))

    iota_mat = np.broadcast_to(np.arange(P, dtype=np.float32), (P, P)).copy()
    W0T = np.ascontiguousarray(W0.T)
    W1T = np.ascontiguousarray(W1.T)
    core_ids = list(range(NCORES))
    LAST_TIMES["prep_s"] = time.perf_counter() - t0

    # ---- launch A: S0 = X @ W0.T, node-sharded ----
    CH = int(np.ceil(N / (NCORES * P)))  # 49 chunks/core
    rows_pc = CH * P
    Xpad = np.zeros((NCORES * rows_pc, D), dtype=np.float32)
    Xpad[:N] = X
    nc_a = _build_program_a(CH)
    in_maps = [{"xin": Xpad[c * rows_pc:(c + 1) * rows_pc], "w0t": W0T}
               for c in range(NCORES)]
    t1 = time.perf_counter()
    res_a = run_bass_kernel_spmd(nc_a, in_maps, core_ids).results
    LAST_TIMES["run_a_s"] = time.perf_counter() - t1
    S0 = np.concatenate([res_a[c]["s0out"] for c in range(NCORES)])[:N]
    S0 = np.ascontiguousarray(S0)

    # ---- launch B: H = relu(seg_sum(S0[src]*norm, dst)) ----
    nc_b = _build_program_bc(G, relu=True, with_w1=False)
    in_maps = [{"srct": S0, "idx": idx_arr[c], "sn": sn_arr[c], "iota": iota_mat}
               for c in range(NCORES)]
    t1 = time.perf_counter()
    res_b = run_bass_kernel_spmd(nc_b, in_maps, core_ids).results
    LAST_TIMES["run_b_s"] = time.perf_counter() - t1
    H = np.zeros((N, D), dtype=np.float32)
    for c in range(NCORES):
        H[asm_ids[c]] = res_b[c]["out"][asm_rows[c]]

    # ---- launch C: Z = seg_sum(H[src]*norm, dst) @ W1.T ----
    nc_c = _build_program_bc(G, relu=False, with_w1=True)
    in_maps = [{"srct": H, "idx": idx_arr[c], "sn": sn_arr[c], "iota": iota_mat,
                "w1t": W1T} for c in range(NCORES)]
    t1 = time.perf_counter()
    res_c = run_bass_kernel_spmd(nc_c, in_maps, core_ids).results
    LAST_TIMES["run_c_s"] = time.perf_counter() - t1
    Z = np.zeros((N, D), dtype=np.float32)
    for c in range(NCORES):
        Z[asm_ids[c]] = res_c[c]["out"][asm_rows[c]]

    LAST_TIMES["total_s"] = time.perf_counter() - t0
    return (Z, H)
